# revision 21
# baseline (speedup 1.0000x reference)
"""BilateralSliceApply kernel for 8 Trainium2 NeuronCores.

Math (from the reference):
  out = a * (x0 + x1 + x2) + b, where (a, b) are the 2 channels of the
  bilateral grid trilinearly sliced at (ix(w), iy(h), iz(guide)).

  iz = (guide + 1) * 0.5 * (gd - 1) = 3.5 * guide + 3.5 in [3.5, 7) since
  guide is in [0, 1). On that range the z interpolation is a piecewise
  linear function of iz with knots at 4, 5, 6, expressible in hinge form:

    coeff(iz) = base + g * E3 + relu(3.5 g - 0.5) * E4
                     + relu(3.5 g - 1.5) * E5 + relu(3.5 g - 2.5) * E6

  where, with Gz[k] the xy-interpolated grid at z-plane k and
  D[k] = Gz[k+1] - Gz[k]:
    base = Gz[3] + 0.5 D[3],  E3 = 3.5 D[3],  Ek = D[k] - D[k-1].

  The xy bilinear interpolation is separable: the y direction (per output
  row) is folded into small host-precomputed row tables; the x direction
  is a K=16 matmul against a hat-function matrix done on the tensor
  engine, producing the 10 "plane" images (5 hinge basis x 2 channels)
  per 128-row block.

Sharding: 8 shards = batch (4) x H-halves (2), one per core.
"""

import sys

sys.path.insert(0, "/opt/trn_rl_repo")

from contextlib import ExitStack

import numpy as np

import concourse.bacc as bacc
import concourse.bass as bass
import concourse.mybir as mybir
from concourse import tile
from concourse.bass_utils import run_bass_kernel_spmd

N, C, GH, GW, GD = 4, 2, 16, 16, 8
H, W = 2048, 2048
N_CORES = 8
ROWS_PER_CORE = H // 2          # shard = (batch, h-half)
BLK_R = 128                     # rows per block
N_RBLK = ROWS_PER_CORE // BLK_R  # 8
COL_W = 1024                    # columns per work item
N_CBLK = W // COL_W             # 2
N_PLANES = 10                   # 5 hinge basis x 2 channels

CHAIN_DT = mybir.dt.float32     # dtype of planes / per-pixel chain
SKIP_CHAIN = False              # diagnostic: skip the DVE chain
N_EVAC_DVE = 5                  # planes evacuated by DVE instead of ACT
_NP_CHAIN = {mybir.dt.float32: np.float32,
             mybir.dt.float16: np.float16,
             mybir.dt.bfloat16: None}[CHAIN_DT]

_NC_CACHE = {}


def _build_nc(repeat=1):
    key = (CHAIN_DT, COL_W, repeat, SKIP_CHAIN, N_EVAC_DVE)
    if key in _NC_CACHE:
        return _NC_CACHE[key]
    f32 = mybir.dt.float32
    cd = CHAIN_DT
    nc = bacc.Bacc("TRN2", target_bir_lowering=False, debug=False,
                   enable_asserts=False, num_devices=N_CORES)
    guide = nc.dram_tensor("guide", [ROWS_PER_CORE, W], f32,
                           kind="ExternalInput").ap()
    xin = nc.dram_tensor("xin", [3, ROWS_PER_CORE, W], f32,
                         kind="ExternalInput").ap()
    bf16 = mybir.dt.bfloat16
    KS = 6 * GW  # split-precision stacked contraction dim = 96
    tabs = nc.dram_tensor("tabs", [N_RBLK, KS, N_PLANES * BLK_R], bf16,
                          kind="ExternalInput").ap()
    rxt = nc.dram_tensor("rxt", [KS, W], bf16, kind="ExternalInput").ap()
    out = nc.dram_tensor("out", [ROWS_PER_CORE, W], f32,
                         kind="ExternalOutput").ap()

    Relu = mybir.ActivationFunctionType.Relu
    mult = mybir.AluOpType.mult
    addo = mybir.AluOpType.add

    with tile.TileContext(nc) as tc:
        with ExitStack() as ctx:
            const_p = ctx.enter_context(tc.tile_pool(name="const", bufs=1))
            tab_p = ctx.enter_context(tc.tile_pool(name="tab", bufs=2))
            g_p = ctx.enter_context(tc.tile_pool(name="g", bufs=2))
            s_p = ctx.enter_context(tc.tile_pool(name="s", bufs=2))
            r_p = ctx.enter_context(tc.tile_pool(name="r", bufs=2))
            pl_p = ctx.enter_context(tc.tile_pool(name="pl", bufs=12))
            ps_p = ctx.enter_context(
                tc.tile_pool(name="ps", bufs=4, space="PSUM"))
            tmp_p = ctx.enter_context(tc.tile_pool(name="tmp", bufs=2))
            acc_p = ctx.enter_context(tc.tile_pool(name="acc", bufs=2))
            out_p = ctx.enter_context(tc.tile_pool(name="o", bufs=2))

            rxt_t = const_p.tile([KS, W], bf16)
            nc.sync.dma_start(rxt_t[:], rxt[:])
            bias_t = {}
            for k in (4, 5, 6):
                bt = const_p.tile([BLK_R, 1], f32, tag=f"bias{k}")
                nc.vector.memset(bt[:], 3.5 - k)
                bias_t[k] = bt

            for rb in [r for _ in range(repeat) for r in range(N_RBLK)]:
                tab_t = tab_p.tile([KS, N_PLANES * BLK_R], bf16, tag="tab")
                nc.sync.dma_start(tab_t[:], tabs[rb])
                r0 = rb * BLK_R
                for cb in range(N_CBLK):
                    c0 = cb * COL_W
                    g_t = g_p.tile([BLK_R, COL_W], f32, tag="g")
                    nc.sync.dma_start(
                        g_t[:], guide[r0:r0 + BLK_R, c0:c0 + COL_W])
                    s_t = s_p.tile([BLK_R, COL_W], f32, tag="s")
                    nc.gpsimd.dma_start(
                        out=s_t[:], in_=xin[0, r0:r0 + BLK_R, c0:c0 + COL_W])
                    for ch in (1, 2):
                        nc.gpsimd.dma_start(
                            out=s_t[:],
                            in_=xin[ch, r0:r0 + BLK_R, c0:c0 + COL_W],
                            accum_op=addo)

                    # r_k = relu(3.5 * g - (k - 3.5)), k = 4, 5, 6
                    rk = []
                    for k in (4, 5, 6):
                        r_t = r_p.tile([BLK_R, COL_W], cd, tag=f"r{k}")
                        nc.scalar.activation(r_t[:], g_t[:], Relu,
                                             bias=bias_t[k][:], scale=3.5)
                        rk.append(r_t)
                    if cd != f32:
                        g_c = r_p.tile([BLK_R, COL_W], cd, tag="gc")
                        nc.vector.tensor_copy(g_c[:], g_t[:])
                        s_c = r_p.tile([BLK_R, COL_W], cd, tag="sc")
                        nc.vector.tensor_copy(s_c[:], s_t[:])
                    else:
                        g_c, s_c = g_t, s_t

                    planes = []
                    for p in range(N_PLANES):
                        ps_t = ps_p.tile([BLK_R, COL_W], f32, tag="ps")
                        for mc in range(COL_W // 512):
                            nc.tensor.matmul(
                                ps_t[:, mc * 512:(mc + 1) * 512],
                                tab_t[:, p * BLK_R:(p + 1) * BLK_R],
                                rxt_t[:, c0 + mc * 512:c0 + (mc + 1) * 512],
                                start=True, stop=True)
                        pl_t = pl_p.tile([BLK_R, COL_W], cd, tag="pl")
                        # interleave ACT/DVE evacuation to balance engines
                        if p % 2 == 0 and p // 2 < N_EVAC_DVE:
                            nc.vector.tensor_copy(pl_t[:], ps_t[:])
                        else:
                            nc.scalar.copy(pl_t[:], ps_t[:])
                        planes.append(pl_t)

                    # coeff chains: planes[0..4] -> a, planes[5..9] -> b
                    # (diagnostic mode shrinks chain op width to isolate
                    #  the DVE chain cost from the rest of the pipeline)
                    cw = 8 if SKIP_CHAIN else COL_W
                    accs = []
                    for cch in range(2):
                        base, e3, e4, e5, e6 = planes[cch * 5:cch * 5 + 5]
                        t0 = tmp_p.tile([BLK_R, COL_W], cd, tag="t0")
                        nc.vector.tensor_mul(t0[:, :cw], g_c[:, :cw],
                                             e3[:, :cw])
                        acc = acc_p.tile([BLK_R, COL_W], cd, tag=f"acc{cch}")
                        nc.vector.tensor_add(acc[:, :cw], base[:, :cw],
                                             t0[:, :cw])
                        for r_t, e_t in zip(rk, (e4, e5, e6)):
                            t1 = tmp_p.tile([BLK_R, COL_W], cd, tag="t1")
                            nc.vector.tensor_mul(t1[:, :cw], r_t[:, :cw],
                                                 e_t[:, :cw])
                            nc.vector.tensor_add(acc[:, :cw], acc[:, :cw],
                                                 t1[:, :cw])
                        accs.append(acc)

                    prod = tmp_p.tile([BLK_R, COL_W], cd, tag="prod")
                    nc.vector.tensor_mul(prod[:, :cw], accs[0][:, :cw],
                                         s_c[:, :cw])
                    o_t = out_p.tile([BLK_R, COL_W], f32, tag="o")
                    nc.vector.tensor_add(o_t[:, :cw], prod[:, :cw],
                                         accs[1][:, :cw])
                    nc.sync.dma_start(
                        out[r0:r0 + BLK_R, c0:c0 + COL_W], o_t[:])
    nc.compile()
    _NC_CACHE[key] = nc
    return nc


def _build_nc_repeat(repeat):
    return _build_nc(repeat=repeat)


def _host_tables(bilateral_grid):
    """Per-(batch, h-half) row tables [N_RBLK, GW, N_PLANES*BLK_R] and the
    shared x-interp matrix rxt [GW, W]."""
    g64 = np.asarray(bilateral_grid, dtype=np.float64)  # [N,C,GH,GW,GD]
    h = np.arange(H)
    iy = h / (H - 1) * (GH - 1)
    y0 = np.clip(np.floor(iy).astype(np.int64), 0, GH - 1)
    y1 = np.clip(y0 + 1, 0, GH - 1)
    fy = iy - y0
    # grow[n, c, h, j, z]
    grow = ((1.0 - fy)[None, None, :, None, None] * g64[:, :, y0, :, :]
            + fy[None, None, :, None, None] * g64[:, :, y1, :, :])
    D = grow[..., 1:] - grow[..., :-1]
    base = grow[..., 3] + 0.5 * D[..., 3]
    e3 = 3.5 * D[..., 3]
    e4 = D[..., 4] - D[..., 3]
    e5 = D[..., 5] - D[..., 4]
    e6 = D[..., 6] - D[..., 5]
    # [n, c, 5, h, j] -> planes p = c*5 + basis
    pt = np.stack([base, e3, e4, e5, e6], axis=2)
    pt = pt.reshape(N, N_PLANES, H, GW)
    # tabs_f32[n, half, rblk, j, p*r]
    pt = pt.transpose(0, 2, 3, 1)                    # [n, h, j, p]
    pt = pt.reshape(N, 2, N_RBLK, BLK_R, GW, N_PLANES)
    tabs_f = pt.transpose(0, 1, 2, 4, 5, 3).reshape(
        N, 2, N_RBLK, GW, N_PLANES * BLK_R).astype(np.float32)

    w = np.arange(W)
    ix = w / (W - 1) * (GW - 1)
    x0 = np.clip(np.floor(ix).astype(np.int64), 0, GW - 1)
    x1 = np.clip(x0 + 1, 0, GW - 1)
    fx = ix - x0
    rxt_f = np.zeros((GW, W))
    rxt_f[x0, w] += 1.0 - fx
    np.add.at(rxt_f, (x1, w), fx)
    rxt_f = rxt_f.astype(np.float32)

    # Split-precision: x = h + m + l with h/m/l bf16; stack the 6 cross
    # terms whose magnitude is >= 2^-18 along K so one bf16 matmul
    # computes a virtually fp32-exact product:
    #   T stack: [Th Th Tm Th Tm Tl],  R stack: [Rh Rm Rh Rl Rm Rh]
    import ml_dtypes
    bf = ml_dtypes.bfloat16

    def split3(x):
        h = x.astype(bf)
        r1 = x - h.astype(np.float32)
        m = r1.astype(bf)
        l = (r1 - m.astype(np.float32)).astype(bf)
        return h, m, l

    th, tm, tl = split3(tabs_f)
    rh, rm, rl = split3(rxt_f)
    tabs = np.concatenate([th, th, tm, th, tm, tl], axis=3)  # [n,2,8,96,1280]
    rxt = np.concatenate([rh, rm, rh, rl, rm, rh], axis=0)   # [96, W]
    return tabs, rxt


def kernel(bilateral_grid, guidemap, full_res_input):
    guidemap = np.ascontiguousarray(np.asarray(guidemap), dtype=np.float32)
    full_res_input = np.ascontiguousarray(
        np.asarray(full_res_input), dtype=np.float32)
    tabs, rxt = _host_tables(bilateral_grid)

    nc = _build_nc()
    in_maps = []
    for core in range(N_CORES):
        n, half = divmod(core, 2)
        r0 = half * ROWS_PER_CORE
        in_maps.append({
            "guide": guidemap[n, r0:r0 + ROWS_PER_CORE],
            "xin": full_res_input[n, :, r0:r0 + ROWS_PER_CORE],
            "tabs": tabs[n, half],
            "rxt": rxt,
        })
    res = run_bass_kernel_spmd(nc, in_maps, list(range(N_CORES)), trace=False)
    out = np.empty((N, 1, H, W), dtype=np.float32)
    for core in range(N_CORES):
        n, half = divmod(core, 2)
        r0 = half * ROWS_PER_CORE
        out[n, 0, r0:r0 + ROWS_PER_CORE] = res.results[core]["out"]
    return out


# revision 27
# speedup vs baseline: 1.7223x; 1.7223x over previous
"""BilateralSliceApply kernel for 8 Trainium2 NeuronCores.

Math (from the reference):
  out = a * (x0 + x1 + x2) + b, where (a, b) are the 2 channels of the
  bilateral grid trilinearly sliced at (ix(w), iy(h), iz(guide)).

  iz = (guide + 1) * 0.5 * (gd - 1) = 3.5 * guide + 3.5 in [3.5, 7) since
  guide is in [0, 1). On that range the z interpolation is a piecewise
  linear function of iz with knots at 4, 5, 6, expressible in hinge form:

    coeff(iz) = base + g * E3 + relu(3.5 g - 0.5) * E4
                     + relu(3.5 g - 1.5) * E5 + relu(3.5 g - 2.5) * E6

  where, with Gz[k] the xy-interpolated grid at z-plane k and
  D[k] = Gz[k+1] - Gz[k]:
    base = Gz[3] + 0.5 D[3],  E3 = 3.5 D[3],  Ek = D[k] - D[k-1].

  The xy bilinear interpolation is separable: the y direction (per output
  row) is folded into small host-precomputed row tables; the x direction
  is a K=16 matmul against a hat-function matrix done on the tensor
  engine, producing the 10 "plane" images (5 hinge basis x 2 channels)
  per 128-row block.

Sharding: 8 shards = batch (4) x H-halves (2), one per core.
"""

import sys

sys.path.insert(0, "/opt/trn_rl_repo")

from contextlib import ExitStack

import numpy as np

import concourse.bacc as bacc
import concourse.bass as bass
import concourse.mybir as mybir
from concourse import tile
from concourse.bass_utils import run_bass_kernel_spmd

N, C, GH, GW, GD = 4, 2, 16, 16, 8
H, W = 2048, 2048
N_CORES = 8
ROWS_PER_CORE = H // 2          # shard = (batch, h-half)
BLK_R = 128                     # rows per block
N_RBLK = ROWS_PER_CORE // BLK_R  # 8
COL_W = 1024                    # columns per work item
N_CBLK = W // COL_W             # 2
N_PLANES = 10                   # 5 hinge basis x 2 channels

CHAIN_DT = mybir.dt.float32     # dtype of planes / per-pixel chain
SKIP_CHAIN = False              # diagnostic: skip the DVE chain
N_EVAC_DVE = 0                  # planes evacuated by DVE instead of ACT
SKIP_EVAC = False               # diagnostic: skip PSUM->SBUF evacuation
_NP_CHAIN = {mybir.dt.float32: np.float32,
             mybir.dt.float16: np.float16,
             mybir.dt.bfloat16: None}[CHAIN_DT]

_NC_CACHE = {}


def _build_nc(repeat=1):
    key = (CHAIN_DT, COL_W, repeat, SKIP_CHAIN, N_EVAC_DVE, SKIP_EVAC)
    if key in _NC_CACHE:
        return _NC_CACHE[key]
    f32 = mybir.dt.float32
    cd = CHAIN_DT
    nc = bacc.Bacc("TRN2", target_bir_lowering=False, debug=False,
                   enable_asserts=False, num_devices=N_CORES)
    guide = nc.dram_tensor("guide", [ROWS_PER_CORE, W], f32,
                           kind="ExternalInput").ap()
    xin = nc.dram_tensor("xin", [3, ROWS_PER_CORE, W], f32,
                         kind="ExternalInput").ap()
    bf16 = mybir.dt.bfloat16
    KS = 6 * GW  # split-precision stacked contraction dim = 96
    tabs = nc.dram_tensor("tabs", [N_RBLK, KS, N_PLANES * BLK_R], bf16,
                          kind="ExternalInput").ap()
    rxt = nc.dram_tensor("rxt", [KS, W], bf16, kind="ExternalInput").ap()
    out = nc.dram_tensor("out", [ROWS_PER_CORE, W], f32,
                         kind="ExternalOutput").ap()

    Relu = mybir.ActivationFunctionType.Relu
    mult = mybir.AluOpType.mult
    addo = mybir.AluOpType.add

    with tile.TileContext(nc) as tc:
        with ExitStack() as ctx:
            const_p = ctx.enter_context(tc.tile_pool(name="const", bufs=1))
            tab_p = ctx.enter_context(tc.tile_pool(name="tab", bufs=2))
            g_p = ctx.enter_context(tc.tile_pool(name="g", bufs=3))
            s_p = ctx.enter_context(tc.tile_pool(name="s", bufs=3))
            r_p = ctx.enter_context(tc.tile_pool(name="r", bufs=2))
            pl_p = ctx.enter_context(tc.tile_pool(name="pl", bufs=7))
            ps_p = ctx.enter_context(
                tc.tile_pool(name="ps", bufs=2, space="PSUM"))
            tmp_p = ctx.enter_context(tc.tile_pool(name="tmp", bufs=2))
            acc_p = ctx.enter_context(tc.tile_pool(name="acc", bufs=2))
            out_p = ctx.enter_context(tc.tile_pool(name="o", bufs=3))

            rxt_t = const_p.tile([KS, W], bf16)
            nc.sync.dma_start(rxt_t[:], rxt[:])
            bias_t = {}
            for k in (4, 5, 6):
                bt = const_p.tile([BLK_R, 1], f32, tag=f"bias{k}")
                nc.vector.memset(bt[:], 3.5 - k)
                bias_t[k] = bt

            for rb in [r for _ in range(repeat) for r in range(N_RBLK)]:
                tab_t = tab_p.tile([KS, N_PLANES * BLK_R], bf16, tag="tab")
                nc.sync.dma_start(tab_t[:], tabs[rb])
                r0 = rb * BLK_R
                for cb in range(N_CBLK):
                    c0 = cb * COL_W
                    g_t = g_p.tile([BLK_R, COL_W], f32, tag="g")
                    nc.sync.dma_start(
                        g_t[:], guide[r0:r0 + BLK_R, c0:c0 + COL_W])
                    s_t = s_p.tile([BLK_R, COL_W], f32, tag="s")
                    nc.gpsimd.dma_start(
                        out=s_t[:], in_=xin[0, r0:r0 + BLK_R, c0:c0 + COL_W])
                    for ch in (1, 2):
                        nc.gpsimd.dma_start(
                            out=s_t[:],
                            in_=xin[ch, r0:r0 + BLK_R, c0:c0 + COL_W],
                            accum_op=addo)

                    # r_k = relu(3.5 * g - (k - 3.5)), k = 4, 5, 6
                    rk = []
                    for k in (4, 5, 6):
                        r_t = r_p.tile([BLK_R, COL_W], cd, tag=f"r{k}")
                        nc.scalar.activation(r_t[:], g_t[:], Relu,
                                             bias=bias_t[k][:], scale=3.5)
                        rk.append(r_t)
                    if cd != f32:
                        g_c = r_p.tile([BLK_R, COL_W], cd, tag="gc")
                        nc.vector.tensor_copy(g_c[:], g_t[:])
                        s_c = r_p.tile([BLK_R, COL_W], cd, tag="sc")
                        nc.vector.tensor_copy(s_c[:], s_t[:])
                    else:
                        g_c, s_c = g_t, s_t

                    # planes are produced (and evacuated) in pairs: one
                    # PSUM tile holds two planes side by side so a single
                    # wide ACT copy evacuates both (halves per-op init).
                    planes = []
                    for q in range(N_PLANES // 2):
                        ps_t = ps_p.tile([BLK_R, 2 * COL_W], f32, tag="ps")
                        for half in range(2):
                            p = 2 * q + half
                            po = half * COL_W
                            for mc in range(COL_W // 512):
                                nc.tensor.matmul(
                                    ps_t[:, po + mc * 512:po + (mc + 1) * 512],
                                    tab_t[:, p * BLK_R:(p + 1) * BLK_R],
                                    rxt_t[:, c0 + mc * 512:c0 + (mc + 1) * 512],
                                    start=True, stop=True)
                        pl_t = pl_p.tile([BLK_R, 2 * COL_W], cd, tag="pl")
                        if SKIP_EVAC:
                            nc.scalar.copy(pl_t[:, :8], ps_t[:, :8])
                        elif q < N_EVAC_DVE:
                            nc.vector.tensor_copy(pl_t[:], ps_t[:])
                        else:
                            nc.scalar.copy(pl_t[:], ps_t[:])
                        planes.append(pl_t[:, 0:COL_W])
                        planes.append(pl_t[:, COL_W:2 * COL_W])

                    # coeff chains: planes[0..4] -> a, planes[5..9] -> b
                    # (diagnostic mode shrinks chain op width to isolate
                    #  the DVE chain cost from the rest of the pipeline)
                    cw = 8 if SKIP_CHAIN else COL_W
                    accs = []
                    for cch in range(2):
                        base, e3, e4, e5, e6 = planes[cch * 5:cch * 5 + 5]
                        t0 = tmp_p.tile([BLK_R, COL_W], cd, tag="t0")
                        nc.vector.tensor_mul(t0[:, :cw], g_c[:, :cw],
                                             e3[:, :cw])
                        acc = acc_p.tile([BLK_R, COL_W], cd, tag=f"acc{cch}")
                        nc.vector.tensor_add(acc[:, :cw], base[:, :cw],
                                             t0[:, :cw])
                        for r_t, e_t in zip(rk, (e4, e5, e6)):
                            t1 = tmp_p.tile([BLK_R, COL_W], cd, tag="t1")
                            nc.vector.tensor_mul(t1[:, :cw], r_t[:, :cw],
                                                 e_t[:, :cw])
                            nc.vector.tensor_add(acc[:, :cw], acc[:, :cw],
                                                 t1[:, :cw])
                        accs.append(acc)

                    prod = tmp_p.tile([BLK_R, COL_W], cd, tag="prod")
                    nc.vector.tensor_mul(prod[:, :cw], accs[0][:, :cw],
                                         s_c[:, :cw])
                    o_t = out_p.tile([BLK_R, COL_W], f32, tag="o")
                    nc.vector.tensor_add(o_t[:, :cw], prod[:, :cw],
                                         accs[1][:, :cw])
                    nc.sync.dma_start(
                        out[r0:r0 + BLK_R, c0:c0 + COL_W], o_t[:])
    nc.compile()
    _NC_CACHE[key] = nc
    return nc


def _build_nc_repeat(repeat):
    return _build_nc(repeat=repeat)


def _host_tables(bilateral_grid):
    """Per-(batch, h-half) row tables [N_RBLK, GW, N_PLANES*BLK_R] and the
    shared x-interp matrix rxt [GW, W]."""
    g64 = np.asarray(bilateral_grid, dtype=np.float64)  # [N,C,GH,GW,GD]
    h = np.arange(H)
    iy = h / (H - 1) * (GH - 1)
    y0 = np.clip(np.floor(iy).astype(np.int64), 0, GH - 1)
    y1 = np.clip(y0 + 1, 0, GH - 1)
    fy = iy - y0
    # grow[n, c, h, j, z]
    grow = ((1.0 - fy)[None, None, :, None, None] * g64[:, :, y0, :, :]
            + fy[None, None, :, None, None] * g64[:, :, y1, :, :])
    D = grow[..., 1:] - grow[..., :-1]
    base = grow[..., 3] + 0.5 * D[..., 3]
    e3 = 3.5 * D[..., 3]
    e4 = D[..., 4] - D[..., 3]
    e5 = D[..., 5] - D[..., 4]
    e6 = D[..., 6] - D[..., 5]
    # [n, c, 5, h, j] -> planes p = c*5 + basis
    pt = np.stack([base, e3, e4, e5, e6], axis=2)
    pt = pt.reshape(N, N_PLANES, H, GW)
    # tabs_f32[n, half, rblk, j, p*r]
    pt = pt.transpose(0, 2, 3, 1)                    # [n, h, j, p]
    pt = pt.reshape(N, 2, N_RBLK, BLK_R, GW, N_PLANES)
    tabs_f = pt.transpose(0, 1, 2, 4, 5, 3).reshape(
        N, 2, N_RBLK, GW, N_PLANES * BLK_R).astype(np.float32)

    w = np.arange(W)
    ix = w / (W - 1) * (GW - 1)
    x0 = np.clip(np.floor(ix).astype(np.int64), 0, GW - 1)
    x1 = np.clip(x0 + 1, 0, GW - 1)
    fx = ix - x0
    rxt_f = np.zeros((GW, W))
    rxt_f[x0, w] += 1.0 - fx
    np.add.at(rxt_f, (x1, w), fx)
    rxt_f = rxt_f.astype(np.float32)

    # Split-precision: x = h + m + l with h/m/l bf16; stack the 6 cross
    # terms whose magnitude is >= 2^-18 along K so one bf16 matmul
    # computes a virtually fp32-exact product:
    #   T stack: [Th Th Tm Th Tm Tl],  R stack: [Rh Rm Rh Rl Rm Rh]
    import ml_dtypes
    bf = ml_dtypes.bfloat16

    def split3(x):
        h = x.astype(bf)
        r1 = x - h.astype(np.float32)
        m = r1.astype(bf)
        l = (r1 - m.astype(np.float32)).astype(bf)
        return h, m, l

    th, tm, tl = split3(tabs_f)
    rh, rm, rl = split3(rxt_f)
    tabs = np.concatenate([th, th, tm, th, tm, tl], axis=3)  # [n,2,8,96,1280]
    rxt = np.concatenate([rh, rm, rh, rl, rm, rh], axis=0)   # [96, W]
    return tabs, rxt


def kernel(bilateral_grid, guidemap, full_res_input):
    guidemap = np.ascontiguousarray(np.asarray(guidemap), dtype=np.float32)
    full_res_input = np.ascontiguousarray(
        np.asarray(full_res_input), dtype=np.float32)
    tabs, rxt = _host_tables(bilateral_grid)

    nc = _build_nc()
    in_maps = []
    for core in range(N_CORES):
        n, half = divmod(core, 2)
        r0 = half * ROWS_PER_CORE
        in_maps.append({
            "guide": guidemap[n, r0:r0 + ROWS_PER_CORE],
            "xin": full_res_input[n, :, r0:r0 + ROWS_PER_CORE],
            "tabs": tabs[n, half],
            "rxt": rxt,
        })
    res = run_bass_kernel_spmd(nc, in_maps, list(range(N_CORES)), trace=False)
    out = np.empty((N, 1, H, W), dtype=np.float32)
    for core in range(N_CORES):
        n, half = divmod(core, 2)
        r0 = half * ROWS_PER_CORE
        out[n, 0, r0:r0 + ROWS_PER_CORE] = res.results[core]["out"]
    return out


# revision 29
# speedup vs baseline: 2.2162x; 1.2868x over previous
"""BilateralSliceApply kernel for 8 Trainium2 NeuronCores.

Math (from the reference):
  out = a * (x0 + x1 + x2) + b, where (a, b) are the 2 channels of the
  bilateral grid trilinearly sliced at (ix(w), iy(h), iz(guide)).

  iz = (guide + 1) * 0.5 * (gd - 1) = 3.5 * guide + 3.5 in [3.5, 7) since
  guide is in [0, 1). On that range the z interpolation is a piecewise
  linear function of iz with knots at 4, 5, 6, expressible in hinge form:

    coeff(iz) = base + g * E3 + relu(3.5 g - 0.5) * E4
                     + relu(3.5 g - 1.5) * E5 + relu(3.5 g - 2.5) * E6

  where, with Gz[k] the xy-interpolated grid at z-plane k and
  D[k] = Gz[k+1] - Gz[k]:
    base = Gz[3] + 0.5 D[3],  E3 = 3.5 D[3],  Ek = D[k] - D[k-1].

  The xy bilinear interpolation is separable: the y direction (per output
  row) is folded into small host-precomputed row tables; the x direction
  is a K=16 matmul against a hat-function matrix done on the tensor
  engine, producing the 10 "plane" images (5 hinge basis x 2 channels)
  per 128-row block.

Sharding: 8 shards = batch (4) x H-halves (2), one per core.
"""

import sys

sys.path.insert(0, "/opt/trn_rl_repo")

from contextlib import ExitStack

import numpy as np

import concourse.bacc as bacc
import concourse.bass as bass
import concourse.mybir as mybir
from concourse import tile
from concourse.bass_utils import run_bass_kernel_spmd

N, C, GH, GW, GD = 4, 2, 16, 16, 8
H, W = 2048, 2048
N_CORES = 8
ROWS_PER_CORE = H // 2          # shard = (batch, h-half)
BLK_R = 128                     # rows per block
N_RBLK = ROWS_PER_CORE // BLK_R  # 8
COL_W = 1024                    # columns per work item
N_CBLK = W // COL_W             # 2
N_PLANES = 10                   # 5 hinge basis x 2 channels

CHAIN_DT = mybir.dt.float32     # dtype of planes / per-pixel chain
SKIP_CHAIN = False              # diagnostic: skip the DVE chain
N_EVAC_DVE = 0                  # planes evacuated by DVE instead of ACT
SKIP_EVAC = False               # diagnostic: skip PSUM->SBUF evacuation
_NP_CHAIN = {mybir.dt.float32: np.float32,
             mybir.dt.float16: np.float16,
             mybir.dt.bfloat16: None}[CHAIN_DT]

_NC_CACHE = {}


def _build_nc(repeat=1):
    key = (CHAIN_DT, COL_W, repeat, SKIP_CHAIN, N_EVAC_DVE, SKIP_EVAC)
    if key in _NC_CACHE:
        return _NC_CACHE[key]
    f32 = mybir.dt.float32
    cd = CHAIN_DT
    nc = bacc.Bacc("TRN2", target_bir_lowering=False, debug=False,
                   enable_asserts=False, num_devices=N_CORES)
    guide = nc.dram_tensor("guide", [ROWS_PER_CORE, W], f32,
                           kind="ExternalInput").ap()
    xin = nc.dram_tensor("xin", [3, ROWS_PER_CORE, W], f32,
                         kind="ExternalInput").ap()
    bf16 = mybir.dt.bfloat16
    KS = 6 * GW  # split-precision stacked contraction dim = 96
    tabs = nc.dram_tensor("tabs", [N_RBLK, KS, N_PLANES * BLK_R], bf16,
                          kind="ExternalInput").ap()
    rxt = nc.dram_tensor("rxt", [KS, W], bf16, kind="ExternalInput").ap()
    out = nc.dram_tensor("out", [ROWS_PER_CORE, W], f32,
                         kind="ExternalOutput").ap()

    Relu = mybir.ActivationFunctionType.Relu
    mult = mybir.AluOpType.mult
    addo = mybir.AluOpType.add

    with tile.TileContext(nc) as tc:
        with ExitStack() as ctx:
            const_p = ctx.enter_context(tc.tile_pool(name="const", bufs=1))
            tab_p = ctx.enter_context(tc.tile_pool(name="tab", bufs=2))
            g_p = ctx.enter_context(tc.tile_pool(name="g", bufs=3))
            s_p = ctx.enter_context(tc.tile_pool(name="s", bufs=3))
            r_p = ctx.enter_context(tc.tile_pool(name="r", bufs=2))
            pl_p = ctx.enter_context(tc.tile_pool(name="pl", bufs=7))
            ps_p = ctx.enter_context(
                tc.tile_pool(name="ps", bufs=2, space="PSUM"))
            tmp_p = ctx.enter_context(tc.tile_pool(name="tmp", bufs=2))
            acc_p = ctx.enter_context(tc.tile_pool(name="acc", bufs=2))
            out_p = ctx.enter_context(tc.tile_pool(name="o", bufs=3))

            rxt_t = const_p.tile([KS, W], bf16)
            nc.sync.dma_start(rxt_t[:], rxt[:])
            bias_t = {}
            for k in (4, 5, 6):
                bt = const_p.tile([BLK_R, 1], f32, tag=f"bias{k}")
                nc.vector.memset(bt[:], 3.5 - k)
                bias_t[k] = bt

            for rb in [r for _ in range(repeat) for r in range(N_RBLK)]:
                tab_t = tab_p.tile([KS, N_PLANES * BLK_R], bf16, tag="tab")
                nc.sync.dma_start(tab_t[:], tabs[rb])
                r0 = rb * BLK_R
                for cb in range(N_CBLK):
                    c0 = cb * COL_W
                    g_t = g_p.tile([BLK_R, COL_W], f32, tag="g")
                    nc.sync.dma_start(
                        g_t[:], guide[r0:r0 + BLK_R, c0:c0 + COL_W])
                    s_t = s_p.tile([BLK_R, COL_W], f32, tag="s")
                    nc.gpsimd.dma_start(
                        out=s_t[:], in_=xin[0, r0:r0 + BLK_R, c0:c0 + COL_W])
                    for ch in (1, 2):
                        nc.gpsimd.dma_start(
                            out=s_t[:],
                            in_=xin[ch, r0:r0 + BLK_R, c0:c0 + COL_W],
                            accum_op=addo)

                    # r_k = max(g - (k - 3.5)/3.5, 0), k = 4, 5, 6 (the 3.5
                    # slope is folded into the host tables); tensor_scalar
                    # runs 2x_2p on DVE, keeping ACT free for evacuation
                    rk = []
                    for k in (4, 5, 6):
                        r_t = r_p.tile([BLK_R, COL_W], cd, tag=f"r{k}")
                        nc.vector.tensor_scalar(
                            r_t[:], g_t[:], (k - 3.5) / 3.5, 0.0,
                            mybir.AluOpType.subtract, mybir.AluOpType.max)
                        rk.append(r_t)
                    if cd != f32:
                        g_c = r_p.tile([BLK_R, COL_W], cd, tag="gc")
                        nc.vector.tensor_copy(g_c[:], g_t[:])
                        s_c = r_p.tile([BLK_R, COL_W], cd, tag="sc")
                        nc.vector.tensor_copy(s_c[:], s_t[:])
                    else:
                        g_c, s_c = g_t, s_t

                    # planes are produced (and evacuated) in pairs: one
                    # PSUM tile holds two planes side by side so a single
                    # wide ACT copy evacuates both (halves per-op init).
                    planes = []
                    for q in range(N_PLANES // 2):
                        ps_t = ps_p.tile([BLK_R, 2 * COL_W], f32, tag="ps")
                        for half in range(2):
                            p = 2 * q + half
                            po = half * COL_W
                            for mc in range(COL_W // 512):
                                nc.tensor.matmul(
                                    ps_t[:, po + mc * 512:po + (mc + 1) * 512],
                                    tab_t[:, p * BLK_R:(p + 1) * BLK_R],
                                    rxt_t[:, c0 + mc * 512:c0 + (mc + 1) * 512],
                                    start=True, stop=True)
                        pl_t = pl_p.tile([BLK_R, 2 * COL_W], cd, tag="pl")
                        if SKIP_EVAC:
                            nc.scalar.copy(pl_t[:, :8], ps_t[:, :8])
                        elif q < N_EVAC_DVE:
                            nc.vector.tensor_copy(pl_t[:], ps_t[:])
                        else:
                            nc.scalar.copy(pl_t[:], ps_t[:])
                        planes.append(pl_t[:, 0:COL_W])
                        planes.append(pl_t[:, COL_W:2 * COL_W])

                    # coeff chains: planes[0..4] -> a, planes[5..9] -> b
                    # (diagnostic mode shrinks chain op width to isolate
                    #  the DVE chain cost from the rest of the pipeline)
                    cw = 8 if SKIP_CHAIN else COL_W
                    accs = []
                    for cch in range(2):
                        base, e3, e4, e5, e6 = planes[cch * 5:cch * 5 + 5]
                        t0 = tmp_p.tile([BLK_R, COL_W], cd, tag="t0")
                        nc.vector.tensor_mul(t0[:, :cw], g_c[:, :cw],
                                             e3[:, :cw])
                        acc = acc_p.tile([BLK_R, COL_W], cd, tag=f"acc{cch}")
                        nc.vector.tensor_add(acc[:, :cw], base[:, :cw],
                                             t0[:, :cw])
                        for r_t, e_t in zip(rk, (e4, e5, e6)):
                            t1 = tmp_p.tile([BLK_R, COL_W], cd, tag="t1")
                            nc.vector.tensor_mul(t1[:, :cw], r_t[:, :cw],
                                                 e_t[:, :cw])
                            nc.vector.tensor_add(acc[:, :cw], acc[:, :cw],
                                                 t1[:, :cw])
                        accs.append(acc)

                    prod = tmp_p.tile([BLK_R, COL_W], cd, tag="prod")
                    nc.vector.tensor_mul(prod[:, :cw], accs[0][:, :cw],
                                         s_c[:, :cw])
                    o_t = out_p.tile([BLK_R, COL_W], f32, tag="o")
                    nc.vector.tensor_add(o_t[:, :cw], prod[:, :cw],
                                         accs[1][:, :cw])
                    nc.sync.dma_start(
                        out[r0:r0 + BLK_R, c0:c0 + COL_W], o_t[:])
    nc.compile()
    _NC_CACHE[key] = nc
    return nc


def _build_nc_repeat(repeat):
    return _build_nc(repeat=repeat)


def _host_tables(bilateral_grid):
    """Per-(batch, h-half) row tables [N_RBLK, GW, N_PLANES*BLK_R] and the
    shared x-interp matrix rxt [GW, W]."""
    g64 = np.asarray(bilateral_grid, dtype=np.float64)  # [N,C,GH,GW,GD]
    h = np.arange(H)
    iy = h / (H - 1) * (GH - 1)
    y0 = np.clip(np.floor(iy).astype(np.int64), 0, GH - 1)
    y1 = np.clip(y0 + 1, 0, GH - 1)
    fy = iy - y0
    # grow[n, c, h, j, z]
    grow = ((1.0 - fy)[None, None, :, None, None] * g64[:, :, y0, :, :]
            + fy[None, None, :, None, None] * g64[:, :, y1, :, :])
    D = grow[..., 1:] - grow[..., :-1]
    base = grow[..., 3] + 0.5 * D[..., 3]
    e3 = 3.5 * D[..., 3]
    e4 = 3.5 * (D[..., 4] - D[..., 3])
    e5 = 3.5 * (D[..., 5] - D[..., 4])
    e6 = 3.5 * (D[..., 6] - D[..., 5])
    # [n, c, 5, h, j] -> planes p = c*5 + basis
    pt = np.stack([base, e3, e4, e5, e6], axis=2)
    pt = pt.reshape(N, N_PLANES, H, GW)
    # tabs_f32[n, half, rblk, j, p*r]
    pt = pt.transpose(0, 2, 3, 1)                    # [n, h, j, p]
    pt = pt.reshape(N, 2, N_RBLK, BLK_R, GW, N_PLANES)
    tabs_f = pt.transpose(0, 1, 2, 4, 5, 3).reshape(
        N, 2, N_RBLK, GW, N_PLANES * BLK_R).astype(np.float32)

    w = np.arange(W)
    ix = w / (W - 1) * (GW - 1)
    x0 = np.clip(np.floor(ix).astype(np.int64), 0, GW - 1)
    x1 = np.clip(x0 + 1, 0, GW - 1)
    fx = ix - x0
    rxt_f = np.zeros((GW, W))
    rxt_f[x0, w] += 1.0 - fx
    np.add.at(rxt_f, (x1, w), fx)
    rxt_f = rxt_f.astype(np.float32)

    # Split-precision: x = h + m + l with h/m/l bf16; stack the 6 cross
    # terms whose magnitude is >= 2^-18 along K so one bf16 matmul
    # computes a virtually fp32-exact product:
    #   T stack: [Th Th Tm Th Tm Tl],  R stack: [Rh Rm Rh Rl Rm Rh]
    import ml_dtypes
    bf = ml_dtypes.bfloat16

    def split3(x):
        h = x.astype(bf)
        r1 = x - h.astype(np.float32)
        m = r1.astype(bf)
        l = (r1 - m.astype(np.float32)).astype(bf)
        return h, m, l

    th, tm, tl = split3(tabs_f)
    rh, rm, rl = split3(rxt_f)
    tabs = np.concatenate([th, th, tm, th, tm, tl], axis=3)  # [n,2,8,96,1280]
    rxt = np.concatenate([rh, rm, rh, rl, rm, rh], axis=0)   # [96, W]
    return tabs, rxt


def kernel(bilateral_grid, guidemap, full_res_input):
    guidemap = np.ascontiguousarray(np.asarray(guidemap), dtype=np.float32)
    full_res_input = np.ascontiguousarray(
        np.asarray(full_res_input), dtype=np.float32)
    tabs, rxt = _host_tables(bilateral_grid)

    nc = _build_nc()
    in_maps = []
    for core in range(N_CORES):
        n, half = divmod(core, 2)
        r0 = half * ROWS_PER_CORE
        in_maps.append({
            "guide": guidemap[n, r0:r0 + ROWS_PER_CORE],
            "xin": full_res_input[n, :, r0:r0 + ROWS_PER_CORE],
            "tabs": tabs[n, half],
            "rxt": rxt,
        })
    res = run_bass_kernel_spmd(nc, in_maps, list(range(N_CORES)), trace=False)
    out = np.empty((N, 1, H, W), dtype=np.float32)
    for core in range(N_CORES):
        n, half = divmod(core, 2)
        r0 = half * ROWS_PER_CORE
        out[n, 0, r0:r0 + ROWS_PER_CORE] = res.results[core]["out"]
    return out
